# revision 9
# baseline (speedup 1.0000x reference)
"""CenterLoss update kernel for Trainium2, 8-core SPMD — class-sharded.

Reference computation (N=16384 samples, C=10000 classes, D=128 dims):
    embeded_labels = labels @ center          # [N,D] gather via one-hot
    diff = embeded_labels - embeded_preds
    grad = (labels.T @ diff) / (counts + 1)   # counts = labels.T @ ones
    out  = center - 0.5 * grad

Because each row of ``labels`` is one-hot, ``labels.T @ labels == diag(counts)``,
so the whole thing collapses to a single pass over ``labels``:

    S      = labels.T @ embeded_preds         # [C,D] per-class sum of preds
    counts = column sums of labels            # [C]
    out    = beta * center + gamma * S
             beta  = 1 - 0.5*counts/(counts+1)
             gamma = 0.5/(counts+1)

Sharding: classes (columns of labels) are sharded across the 8 cores.  Each
core streams its own [N, C/8] column block of labels through the PE exactly
once as the moving matmul operand, accumulating S.T = preds.T @ labels in a
single PSUM region over all 128 k-tiles, with per-partition partial counts
accumulated on the vector engine (two interleaved accumulators to halve the
serial dependence chain) and reduced by one final PE pass against a ones
vector.  Every core computes its C/8 output shard entirely locally — there is
no inter-core collective at all, so nothing serializes behind the stream.

k-tiles are "virtual": tile q covers sample rows {q + 128*p}.  With that row
order, the stationary preds tiles are exactly contiguous column slices of
preds viewed as [128, N*D/128] row-major — so preds loads as a handful of
large contiguous SWDGE cast-DMAs (fp32 -> fp32r) instead of thousands of
512-byte descriptors, and the labels tile for k-tile q is the strided row
slice labels[q::128, :] (5 KB per partition line, full HWDGE rate).

Labels tiles alternate between the two physical HWDGE rings (sync / scalar)
so both descriptor generators feed the 16 SDMA engines.
"""

import numpy as np

N, C, D = 16384, 10000, 128
NCORES = 8
CS = C // NCORES   # 1250 classes per core
LR = 0.5
P = 128
KT = N // P        # 128 virtual k-tiles
# preds load chunks (columns of the [128, N] natural view): equal 0.5 MB
# chunks keep the SWDGE delivery ahead of the PE's tile consumption (big
# trailing chunks arrive too late and stall the PE mid-stream).
PCHUNKS = [1024] * 16
assert sum(PCHUNKS) == KT * D


def _chunks(width, step=512):
    out = []
    c0 = 0
    while c0 < width:
        out.append((c0, min(step, width - c0)))
        c0 += step
    return out


def build_program(cs=CS, d=D, kt=KT):
    """Build the SPMD Bass program (identical on every core)."""
    import concourse.bacc as bacc
    import concourse.mybir as mybir
    import concourse.tile as tile
    from concourse.masks import make_identity

    f32 = mybir.dt.float32
    f32r = mybir.dt.float32r
    mult = mybir.AluOpType.mult
    add = mybir.AluOpType.add

    n = kt * P
    nt3 = (cs + P - 1) // P  # output tiles over the class shard
    assert cs * 4 <= 3 * 2048, "S.T PSUM tile must fit in 3 banks"

    nc = bacc.Bacc(
        "TRN2",
        target_bir_lowering=False,
        debug=False,
        num_devices=NCORES,
    )

    # preds in its natural [128, n] row-major view: partition p holds rows
    # [128p, 128p+128); column block [128q, 128q+128) is then exactly the
    # stationary tile for virtual k-tile q (rows q+128p on partition p).
    preds = nc.dram_tensor("preds", [P, kt * d], f32, kind="ExternalInput").ap()
    # labels are one-hot 0/1: declaring them float32r (same bits, trivially
    # rounded) lets plain HWDGE DMAs feed fp32r matmuls at full speed.
    labels = nc.dram_tensor("labels", [n, cs], f32r, kind="ExternalInput").ap()
    center = nc.dram_tensor("center", [cs, d], f32, kind="ExternalInput").ap()
    out = nc.dram_tensor("out", [cs, d], f32, kind="ExternalOutput").ap()

    with tile.TileContext(nc) as tc:
        with tc.tile_pool(name="const", bufs=1) as const_pool:
            # stationary preds, fp32r single pass (tolerance is 2e-2; the
            # fp32r rounding error on sums of <=~10 preds is ~1e-3 abs).
            # Separate tiles per chunk so each matmul only waits for its own
            # chunk, not the whole 8 MB load.
            preds_hi = []
            pstart = []
            pofs = 0
            for cch, pw in enumerate(PCHUNKS):
                t = const_pool.tile([P, pw], f32r, name=f"preds_hi_{cch}")
                preds_hi.append(t)
                pstart.append(pofs)
                # SWDGE cast-DMA rounds fp32 -> fp32r
                nc.gpsimd.dma_start(out=t[:], in_=preds[:, pofs:pofs + pw])
                pofs += pw

            # center shard, as nt3 [class, d] tiles (class on partitions);
            # rides the otherwise-idle SWDGE ring, done long before phase 3.
            ctr_sb = const_pool.tile([P, nt3 * d], f32, name="ctr_sb")
            for tt in range(nt3):
                w = min(P, cs - tt * P)
                nc.gpsimd.dma_start(
                    out=ctr_sb[0:w, tt * d:tt * d + d],
                    in_=center[tt * P:tt * P + w, :],
                )

            # identity (phase 3 only) is built on gpsimd — AFTER the SWDGE
            # dispatches above so Q7 work never delays the preds load.
            identity = const_pool.tile([P, P], f32, name="identity")
            make_identity(nc, identity[:])
            ones_col = const_pool.tile([P, 1], f32, name="ones_col")
            nc.vector.memset(ones_col[:], 1.0)

            # per-partition partial counts, accumulated on DVE; two
            # accumulators (even/odd tiles) halve the serial add chain.
            counts_a = const_pool.tile([P, cs], f32, name="counts_a")
            counts_b = const_pool.tile([P, cs], f32, name="counts_b")

            st_sb = const_pool.tile([d, cs], f32, name="st_sb")
            cnt_row = const_pool.tile([1, cs], f32, name="cnt_row")

            # ---------------- phase 1: stream labels ----------------
            with (
                tc.tile_pool(name="lab", bufs=8) as lab_pool,
                tc.tile_pool(name="psum1", bufs=1, space="PSUM") as psum1,
            ):
                st_psum = psum1.tile([d, cs], f32, name="st_psum", space="PSUM")
                for q in range(kt):
                    lab_t = lab_pool.tile([P, cs], f32r, name=f"lab_{q}", tag="lab")
                    eng = nc.sync if q % 2 == 0 else nc.scalar
                    eng.dma_start(out=lab_t[:], in_=labels[q::P, :])
                    col = q * d
                    cch = max(i for i in range(len(PCHUNKS)) if pstart[i] <= col)
                    pch = preds_hi[cch]
                    pc0 = col - pstart[cch]
                    for c0, w in _chunks(cs):
                        nc.tensor.matmul(
                            out=st_psum[:, c0:c0 + w],
                            lhsT=pch[:, pc0:pc0 + d],
                            rhs=lab_t[:, c0:c0 + w],
                            start=(q == 0),
                            stop=(q == kt - 1),
                        )
                    acc = counts_a if q % 2 == 0 else counts_b
                    if q < 2:
                        nc.vector.tensor_copy(out=acc[:], in_=lab_t[:].bitcast(f32))
                    else:
                        nc.vector.tensor_add(
                            out=acc[:], in0=acc[:], in1=lab_t[:].bitcast(f32)
                        )

                # reduce the 2x128 partial count rows with one ones-matmul
                # pass (accumulating both accumulators into the same PSUM)
                cnt_psum = psum1.tile([1, cs], f32, name="cnt_psum", space="PSUM")
                for c0, w in _chunks(cs):
                    nc.tensor.matmul(
                        out=cnt_psum[0:1, c0:c0 + w],
                        lhsT=ones_col[:],
                        rhs=counts_a[:, c0:c0 + w],
                        start=True,
                        stop=False,
                    )
                    nc.tensor.matmul(
                        out=cnt_psum[0:1, c0:c0 + w],
                        lhsT=ones_col[:],
                        rhs=counts_b[:, c0:c0 + w],
                        start=False,
                        stop=True,
                    )
                # cnt first: it gates the PE count-transposes below
                nc.scalar.copy(out=cnt_row[:], in_=cnt_psum[:])
                nc.scalar.copy(out=st_sb[:], in_=st_psum[:])

            # ---------------- phase 3: elementwise update, all local -------
            # counts for all nt3 class tiles land as columns of one [P, nt3]
            # PSUM tile, so beta/gamma come from 5 batched DVE ops instead of
            # 4 tiny ops per tile.
            with (
                tc.tile_pool(name="p3", bufs=2) as p3,
                tc.tile_pool(name="psum3", bufs=1, space="PSUM") as psum3,
            ):
                cnt_all = psum3.tile([P, nt3], f32, name="cnt_all", space="PSUM")
                for tt in range(nt3):
                    w = min(P, cs - tt * P)
                    nc.tensor.transpose(
                        out=cnt_all[0:w, tt:tt + 1],
                        in_=cnt_row[0:1, tt * P:tt * P + w],
                        identity=identity[0:1, 0:1],
                    )
                den = p3.tile([P, nt3], f32, name="den", tag="den", bufs=1)
                nc.vector.tensor_scalar_add(
                    out=den[:], in0=cnt_all[:], scalar1=1.0
                )
                rec = p3.tile([P, nt3], f32, name="rec", tag="rec", bufs=1)
                nc.vector.reciprocal(out=rec[:], in_=den[:])
                gam = p3.tile([P, nt3], f32, name="gam", tag="gam", bufs=1)
                nc.vector.tensor_scalar_mul(out=gam[:], in0=rec[:], scalar1=0.5)
                bet = p3.tile([P, nt3], f32, name="bet", tag="bet", bufs=1)
                nc.vector.tensor_tensor(
                    out=bet[:], in0=cnt_all[:], in1=rec[:], op=mult
                )
                nc.vector.tensor_scalar(
                    out=bet[:], in0=bet[:],
                    scalar1=-0.5, scalar2=1.0, op0=mult, op1=add,
                )

                for tt in range(nt3):
                    w = min(P, cs - tt * P)
                    trp = psum3.tile([P, d], f32, name=f"trp_{tt}", tag="trp",
                                     bufs=3, space="PSUM")
                    nc.tensor.transpose(
                        out=trp[0:w, 0:d],
                        in_=st_sb[:, tt * P:tt * P + w],
                        identity=identity[:, 0:d],
                    )
                    o1 = p3.tile([P, d], f32, name=f"o1_{tt}", tag="o1")
                    nc.vector.tensor_scalar_mul(
                        out=o1[0:w, :], in0=ctr_sb[0:w, tt * d:tt * d + d],
                        scalar1=bet[0:w, tt:tt + 1],
                    )
                    ou = p3.tile([P, d], f32, name=f"ou_{tt}", tag="ou")
                    nc.vector.scalar_tensor_tensor(
                        out=ou[0:w, :], in0=trp[0:w, 0:d],
                        scalar=gam[0:w, tt:tt + 1],
                        in1=o1[0:w, :], op0=mult, op1=add,
                    )
                    eng = nc.sync if tt % 2 == 0 else nc.scalar
                    eng.dma_start(
                        out=out[tt * P:tt * P + w, :], in_=ou[0:w, 0:d]
                    )

    nc.compile()
    return nc


_PROGRAM = None
LAST_RESULTS = None  # BassKernelResults from the most recent run (for test.py)


def _get_program():
    global _PROGRAM
    if _PROGRAM is None:
        _PROGRAM = build_program()
    return _PROGRAM


def kernel(embeded_preds, labels, center):
    from concourse.bass_utils import run_bass_kernel_spmd

    global LAST_RESULTS
    preds = np.ascontiguousarray(np.asarray(embeded_preds, dtype=np.float32))
    lab = np.ascontiguousarray(np.asarray(labels, dtype=np.float32))
    ctr = np.ascontiguousarray(np.asarray(center, dtype=np.float32))
    assert preds.shape == (N, D) and lab.shape == (N, C) and ctr.shape == (C, D)

    nc = _get_program()
    preds_nat = preds.reshape(P, KT * D)  # free view; bytes unchanged
    in_maps = [
        {
            "preds": preds_nat,
            "labels": np.ascontiguousarray(lab[:, j * CS:(j + 1) * CS]),
            "center": np.ascontiguousarray(ctr[j * CS:(j + 1) * CS]),
        }
        for j in range(NCORES)
    ]
    res = run_bass_kernel_spmd(nc, in_maps, core_ids=list(range(NCORES)))
    LAST_RESULTS = res
    return np.concatenate([res.results[j]["out"] for j in range(NCORES)], axis=0)
